# revision 1
# baseline (speedup 1.0000x reference)
"""Trainium2 Bass kernel for nn_CrossAttention (masked+distance-modulated cross attention).

Sharding: 8 cores = batch(2) x head-groups(4). Each core computes 4 of 16 heads
for one batch element, producing a partial output projection; partials are
summed on host (Wp is row-sharded by head).

Per-core dataflow (all activations "feature-major" [C, T]; every DRAM input is
pre-tiled on host into the exact SBUF layout so every DMA is one contiguous
slab — no strided descriptors):
  K^T = (Wk^T x_r^T)              [256, 2048]  f16 in / f32r out, bias via K=1 ones-row
  Q^T = (Wq^T x_q^T)              [256, 2048]  (t1-range-major so attention can start early)
  V   = sum_i y_i^T' Wv_i + bv    [2048, 256]  token-major, per-s-chunk accumulation
  per (t1-range 512, s-chunk 128, head):
    S^T = K^T(chunk)^T Q^T(range) [128s, 512t] f32r (1/sqrt(hd) folded into Wq)
    E   = exp(S^T)                ACT, psum -> f16 sbuf
    P1  = E * (mask*exp(-(d/g)^2))^T   numerator weights  (DVE f16)
    P2  = E * mask^T                   denominator terms  (DVE/GPSIMD split)
    O^T[h] += V[chunk,h]^T P1     f16 matmul on PE cols (0 | 64)
    den[h] += ones^T P2           f16 matmul, concurrent on a spare PE col group
  O^T /= den  (recip + rank-1 f32r matmul broadcast + DVE mul) -> f16
  Z = O^T^T Wp + (bp on host)     f16 matmul -> psum -> sbuf -> DMA out (pre-tiled)
"""

import sys

sys.path.insert(0, "/opt/trn_rl_repo")

import numpy as np

import concourse.bass as bass
import concourse.mybir as mybir
import concourse.tile as tile
from concourse import bacc
from concourse.bass import ts
from concourse.bass_utils import run_bass_kernel_spmd

F32 = mybir.dt.float32
F32R = mybir.dt.float32r
F16 = mybir.dt.float16
Exp = mybir.ActivationFunctionType.Exp

# problem dims (hardcoded per contract)
B, T1, T2, C, NH, NI = 2, 2048, 2048, 1024, 16, 3
GAMMA = 0.5
NCORES = 8
HG = 4            # head groups (cores per batch)
HPG = NH // HG    # heads per group = 4
HD = C // NH      # 64
W = HPG * HD      # local width = 256

# P2 mul runs on GPSIMD for 7/16 of s-chunks (evenly spread)


def build_core_program(t1=T1, t2=T2, c=C, dump=False):
    """One core's program: batch b, head-group hg; all sharding via input data."""
    nc = bacc.Bacc(None, target_bir_lowering=False, debug=False)

    ck_n = c // 128          # contraction chunks for projections
    jk_n = W // 128          # output-partition chunks for Q^T/K^T
    tr_n = t1 // 512         # t1 ranges
    sc_n = t2 // 128         # s chunks
    tc_n = t1 // 128         # t chunks for Z
    er_n = c // 512          # output column ranges for Z

    # pre-tiled inputs (host produces these exact layouts; all DMAs contiguous)
    xq = nc.declare_dram_parameter("xqT", [2, 128, ck_n, t1 // 2], F16, isOutput=False)
    xr = nc.declare_dram_parameter("xrT", [2, 128, ck_n, t2 // 2], F16, isOutput=False)
    yt = nc.declare_dram_parameter("yT", [sc_n, 128, NI, ck_n, 128], F16, isOutput=False)
    mfT = nc.declare_dram_parameter("mfT", [tr_n, 128, sc_n, 512], F16, isOutput=False)
    ffT = nc.declare_dram_parameter("ffT", [tr_n, 128, sc_n, 512], F16, isOutput=False)
    wq = nc.declare_dram_parameter("wq", [128, ck_n, jk_n, 128], F16, isOutput=False)
    wk = nc.declare_dram_parameter("wk", [128, ck_n, jk_n, 128], F16, isOutput=False)
    wv = nc.declare_dram_parameter("wv", [128, NI, ck_n, W], F16, isOutput=False)
    wp = nc.declare_dram_parameter("wp", [128, jk_n, c], F16, isOutput=False)
    bq = nc.declare_dram_parameter("bq", [1, W], F16, isOutput=False)
    bk = nc.declare_dram_parameter("bk", [1, W], F16, isOutput=False)
    bvs = nc.declare_dram_parameter("bvs", [1, W], F16, isOutput=False)
    ones_r = nc.declare_dram_parameter("ones_r", [1, 512], F16, isOutput=False)
    ones_c = nc.declare_dram_parameter("ones_c", [128, 1], F16, isOutput=False)
    ones_b = nc.declare_dram_parameter("ones_b", [128, 128], F32R, isOutput=False)
    zp = nc.declare_dram_parameter("zpart", [tc_n, er_n, 128, 512], F32, isOutput=True)
    if dump:
        qT_d = nc.declare_dram_parameter("qT_d", [128, jk_n, t1], F32R, isOutput=True)
        kT_d = nc.declare_dram_parameter("kT_d", [128, jk_n, t2], F32R, isOutput=True)
        v_d = nc.declare_dram_parameter("v_d", [128, sc_n, W], F16, isOutput=True)
        oT_d = nc.declare_dram_parameter("oT_d", [128, jk_n, t1], F16, isOutput=True)
        e_d = nc.declare_dram_parameter("e_d", [HPG, 128, 512], F16, isOutput=True)
        p1_d = nc.declare_dram_parameter("p1_d", [HPG, 128, 512], F16, isOutput=True)
        rb_d = nc.declare_dram_parameter("rb_d", [HPG, 128, 512], F32, isOutput=True)
        dr_d = nc.declare_dram_parameter("dr_d", [HPG, 128, 512], F32, isOutput=True)

    with tile.TileContext(nc) as tc_:
        with (
            tc_.tile_pool(name="persist", bufs=1) as pers,
        ):
            # ---- constants / weights resident in SBUF ----
            wq_sb = pers.tile([128, ck_n, jk_n, 128], F16, tag="wq")
            wk_sb = pers.tile([128, ck_n, jk_n, 128], F16, tag="wk")
            wv_sb = pers.tile([128, NI, ck_n, W], F16, tag="wv")
            wp_sb = pers.tile([128, jk_n, c], F16, tag="wp")
            bq_sb = pers.tile([1, W], F16, tag="bq")
            bk_sb = pers.tile([1, W], F16, tag="bk")
            bvs_sb = pers.tile([1, W], F16, tag="bvs")
            ones_r_sb = pers.tile([1, 512], F16, tag="ones_r")
            ones_c_sb = pers.tile([128, 1], F16, tag="ones_c")
            ones_b_sb = pers.tile([128, 128], F32R, tag="ones_b")

            # K-projection dependencies first: the DMA queue is FIFO
            nc.sync.dma_start(wk_sb[:], wk[:])
            nc.sync.dma_start(bk_sb[:], bk[:])
            nc.sync.dma_start(ones_r_sb[:], ones_r[:])
            nc.sync.dma_start(wq_sb[:], wq[:])
            nc.sync.dma_start(bq_sb[:], bq[:])
            nc.sync.dma_start(wv_sb[:], wv[:])
            nc.sync.dma_start(bvs_sb[:], bvs[:])
            nc.sync.dma_start(ones_c_sb[:], ones_c[:])
            nc.sync.dma_start(ones_b_sb[:], ones_b[:])
            nc.sync.dma_start(wp_sb[:], wp[:])

            # ---- persistent activations ----
            qT = pers.tile([128, jk_n, t1], F32R, tag="qT")
            kT = pers.tile([128, jk_n, t2], F32R, tag="kT")
            v_sb = pers.tile([128, sc_n, W], F16, tag="v")
            oT = pers.tile([128, jk_n, t1], F16, tag="oT")

            # ---- phase A1: K^T then Q^T projections (x streamed in halves) ----
            with (
                tc_.tile_pool(name="pp", bufs=2, space="PSUM") as pp,
                tc_.tile_pool(name="xpool", bufs=4) as xpool,
            ):
                for src, wsb, bsb, dst, nt in (
                    (xr, wk_sb, bk_sb, kT, t2),
                    (xq, wq_sb, bq_sb, qT, t1),
                ):
                    for g in range(2):
                        x_sb = xpool.tile([128, ck_n, nt // 2], F16, tag="x")
                        nc.sync.dma_start(x_sb[:], src[g])
                        for trl in range(nt // 1024):
                            tr = g * 2 + trl
                            for jk in range(jk_n):
                                ps = pp.tile([128, 512], F32, tag="ps")
                                for ck in range(ck_n):
                                    nc.tensor.matmul(
                                        ps[:],
                                        wsb[:, ck, jk, :],
                                        x_sb[:, ck, ts(trl, 512)],
                                        start=(ck == 0),
                                        stop=False,
                                    )
                                nc.tensor.matmul(
                                    ps[:],
                                    bsb[0:1, ts(jk, 128)],
                                    ones_r_sb[0:1, :],
                                    start=False,
                                    stop=True,
                                )
                                nc.scalar.copy(dst[:, jk, ts(tr, 512)], ps[:])

            # mask/dist slabs: pool opened early so tr0/tr1 prefetch ahead of V
            mpool_cm = tc_.tile_pool(name="mpool", bufs=2)
            mpool = mpool_cm.__enter__()
            mslabs = {}

            def load_mslab(tr):
                mf_all = mpool.tile([128, sc_n, 512], F16, tag="mfall", name=f"mfall{tr}")
                ff_all = mpool.tile([128, sc_n, 512], F16, tag="ffall", name=f"ffall{tr}")
                nc.sync.dma_start(mf_all[:], mfT[tr])
                nc.sync.dma_start(ff_all[:], ffT[tr])
                mslabs[tr] = (mf_all, ff_all)

            load_mslab(0)
            load_mslab(1)

            # ---- phase B: attention (head pairs), V projection fused into the
            # first pass (each V s-chunk produced just before its first AV use),
            # Z projection interleaved per t1-range ----
            with (
                tc_.tile_pool(name="pv", bufs=2, space="PSUM") as pv,
                tc_.tile_pool(name="ypool", bufs=4) as ypool,
                tc_.tile_pool(name="ps_s", bufs=4, space="PSUM") as ps_s,
                tc_.tile_pool(name="ps_o", bufs=2, space="PSUM") as ps_o,
                tc_.tile_pool(name="attn", bufs=8) as attn,
                tc_.tile_pool(name="norm", bufs=2) as norm,
                tc_.tile_pool(name="zout", bufs=4) as zout,
            ):
                def emit_z(ztr):
                    for tcl in range(4):
                        tcc = ztr * 4 + tcl
                        for er in range(er_n):
                            psz = ps_s.tile([128, 512], F32, tag="s", name="psz")
                            for jk in range(jk_n):
                                nc.tensor.matmul(
                                    psz[:],
                                    oT[:, jk, ts(tcc, 128)],
                                    wp_sb[:, jk, ts(er, 512)],
                                    start=(jk == 0),
                                    stop=(jk == jk_n - 1),
                                )
                            z_t = zout.tile([128, 512], F32, tag="zt")
                            nc.vector.tensor_copy(z_t[:], psz[:])
                            nc.sync.dma_start(zp[tcc, er], z_t[:])

                for tr in range(tr_n):
                    if tr not in mslabs:
                        load_mslab(tr)
                    mf_all, ff_all = mslabs.pop(tr)
                    if tr + 1 < tr_n and tr + 1 not in mslabs:
                        load_mslab(tr + 1)
                    for hp in range(HPG // 2):
                        po = [
                            ps_o.tile([128, 512], F32, tag="po", name=f"po{i}")
                            for i in range(2)
                        ]
                        for sc in range(sc_n):
                            if tr > 0 and hp == 1 and sc == 4:
                                emit_z(tr - 1)
                            if tr == 0 and hp == 0:
                                # produce V[sc] just in time for the AV below
                                ysc = ypool.tile([128, NI, ck_n, 128], F16, tag="y")
                                nc.sync.dma_start(ysc[:], yt[sc])
                                pvt = pv.tile([128, W], F32, tag="pv")
                                for i in range(NI):
                                    for ck in range(ck_n):
                                        nc.tensor.matmul(
                                            pvt[:],
                                            ysc[:, i, ck, :],
                                            wv_sb[:, i, ck, :],
                                            start=(i == 0 and ck == 0),
                                            stop=False,
                                        )
                                nc.tensor.matmul(
                                    pvt[:],
                                    ones_r_sb[0:1, 0:128],
                                    bvs_sb[0:1, :],
                                    start=False,
                                    stop=True,
                                )
                                nc.scalar.copy(v_sb[:, sc, :], pvt[:])
                            # S for both heads back-to-back (disjoint PE row
                            # groups 0:64 / 64:128 -> HW can overlap them)
                            s_pair, e_pair, p1_pair, p2_pair = [], [], [], []
                            for i in range(2):
                                h = 2 * hp + i
                                off = 64 * (h % 2)
                                hk = h // 2
                                s_ps = ps_s.tile([128, 512], F32, tag="s", name=f"s{i}")
                                nc.tensor.matmul(
                                    s_ps[:],
                                    kT[off : off + 64, hk, ts(sc, 128)],
                                    qT[off : off + 64, hk, ts(tr, 512)],
                                    start=True,
                                    stop=True,
                                )
                                s_pair.append(s_ps)
                            for i in range(2):
                                e_t = attn.tile([128, 512], F16, tag="e", name=f"e{i}", bufs=10)
                                nc.scalar.activation(e_t[:], s_pair[i][:], Exp)
                                e_pair.append(e_t)
                            for i in range(2):
                                h = 2 * hp + i
                                p1 = attn.tile([128, 512], F16, tag="p1", name=f"p1_{i}", bufs=10)
                                p2 = attn.tile([128, 512], F16, tag="p2", name=f"p2_{i}")
                                nc.vector.tensor_mul(p1[:], e_pair[i][:], ff_all[:, sc, :])
                                if sc % 16 in (1, 3, 5, 8, 10, 12, 14):
                                    nc.gpsimd.tensor_mul(p2[:], e_pair[i][:], mf_all[:, sc, :])
                                else:
                                    nc.vector.tensor_mul(p2[:], e_pair[i][:], mf_all[:, sc, :])
                                if dump and tr == 0 and sc == 0:
                                    nc.sync.dma_start(e_d[h], e_pair[i][:])
                                    nc.sync.dma_start(p1_d[h], p1[:])
                                p1_pair.append(p1)
                                p2_pair.append(p2)
                            for i in range(2):
                                h = 2 * hp + i
                                col = 64 * i
                                dcol = 64 - col
                                nc.tensor.matmul(
                                    po[i][col : col + 64, :],
                                    v_sb[:, sc, ts(h, 64)],
                                    p1_pair[i][:],
                                    start=(sc == 0),
                                    stop=(sc == sc_n - 1),
                                    tile_position=(0, col),
                                )
                                nc.tensor.matmul(
                                    po[i][dcol : dcol + 1, :],
                                    ones_c_sb[:, 0:1],
                                    p2_pair[i][:],
                                    start=(sc == 0),
                                    stop=(sc == sc_n - 1),
                                    tile_position=(0, dcol),
                                )
                        # normalize the pair's O^T slices for this t1-range
                        for i in range(2):
                            h = 2 * hp + i
                            col = 64 * i
                            dcol = 64 - col
                            den = norm.tile([128, 512], F32R, tag="den")
                            rb = norm.tile([128, 512], F32, tag="rb")
                            nc.scalar.copy(
                                den[dcol : dcol + 1, :], po[i][dcol : dcol + 1, :]
                            )
                            if dump and tr == 0:
                                nc.sync.dma_start(dr_d[h], den[:].bitcast(F32))
                            with nc.allow_low_precision(reason="f32r recip, ~19 bit"):
                                nc.vector.reciprocal(
                                    den[dcol : dcol + 1, :], den[dcol : dcol + 1, :]
                                )
                            # broadcast recip row: ones[1,128]^T @ recip[1,512]
                            pb = ps_s.tile([128, 512], F32, tag="s", name="pb")
                            nc.tensor.matmul(
                                pb[:],
                                ones_b_sb[dcol : dcol + 1, :],
                                den[dcol : dcol + 1, :],
                                start=True,
                                stop=True,
                            )
                            nc.vector.tensor_copy(
                                rb[col : col + 64, :], pb[col : col + 64, :]
                            )
                            if dump and tr == 0:
                                nc.sync.dma_start(rb_d[h], rb[:])
                            nc.vector.tensor_mul(
                                oT[col : col + 64, h // 2, ts(tr, 512)],
                                po[i][col : col + 64, :],
                                rb[col : col + 64, :],
                            )
                # Z projection for the PREVIOUS t1-range was emitted inside this
                # range's first pass (emit_z below) so PE has S/AV work while the
                # previous range's normalization chain drains. Emit the last
                # range's Z here at the end.
                if tr == tr_n - 1:
                    emit_z(tr)

            mpool_cm.__exit__(None, None, None)

            if dump:
                nc.sync.dma_start(qT_d[:], qT[:])
                nc.sync.dma_start(kT_d[:], kT[:])
                nc.sync.dma_start(v_d[:], v_sb[:])
                nc.sync.dma_start(oT_d[:], oT[:])

    nc.compile()
    return nc


_NC = None


def _get_nc():
    global _NC
    if _NC is None:
        _NC = build_core_program()
    return _NC


def make_in_maps(inputs):
    x_q = np.asarray(inputs["x_q"], np.float32)
    x_r = np.asarray(inputs["x_r"], np.float32)
    y = np.asarray(inputs["y"], np.float32)
    mask = np.asarray(inputs["mask"])
    dist = np.asarray(inputs["dist"], np.float32)
    Wq, bq, Wk, bk, Wv, bv, Wp, bp = (
        np.asarray(inputs[k], np.float32)
        for k in ("Wq", "bq", "Wk", "bk", "Wv", "bv", "Wp", "bp")
    )

    s = np.float32(1.0 / np.sqrt(HD))
    ck_n, jk_n, tr_n, sc_n = C // 128, W // 128, T1 // 512, T2 // 128

    per_batch = []
    for b in range(B):
        maskf = (mask[b, 0] != 0).astype(np.float32)  # [T1, T2]
        dmod = np.exp(-np.square(dist[b, 0] / GAMMA)).astype(np.float32)
        # [s, t] tiled as [tr, sc, 128, 512]
        def tile_st(a):
            return np.ascontiguousarray(
                a.T.reshape(sc_n, 128, tr_n, 512).transpose(2, 1, 0, 3)
            ).astype(np.float16)

        mfT_ = tile_st(maskf)
        ffT_ = tile_st(maskf * dmod)
        # x^T [c, t] -> [2, 128, ck, t/2]
        def tile_x(a):
            aT = a.T.reshape(ck_n, 128, 2, a.shape[0] // 2)
            return np.ascontiguousarray(aT.transpose(2, 1, 0, 3)).astype(np.float16)

        xqT_ = tile_x(x_q[b])
        xrT_ = tile_x(x_r[b])
        # y[:, b] [NI, T2, C] -> [sc, 128p(c), NI, ck, 128(s)]
        yb = y[:, b].reshape(NI, sc_n, 128, ck_n, 128)  # i, sc, sl, ck, p
        yT_ = np.ascontiguousarray(yb.transpose(1, 4, 0, 3, 2)).astype(np.float16)
        per_batch.append((xqT_, xrT_, yT_, mfT_, ffT_))

    in_maps = []
    for core in range(NCORES):
        b, hg = divmod(core, HG)
        sl = slice(hg * W, (hg + 1) * W)
        xqT_, xrT_, yT_, mfT_, ffT_ = per_batch[b]
        wq_ = (Wq[:, sl] * s).reshape(ck_n, 128, jk_n, 128).transpose(1, 0, 2, 3)
        wk_ = Wk[:, sl].reshape(ck_n, 128, jk_n, 128).transpose(1, 0, 2, 3)
        wv_ = Wv[:, :, sl].reshape(NI, ck_n, 128, W).transpose(2, 0, 1, 3)
        wp_ = Wp[sl, :].reshape(jk_n, 128, C).transpose(1, 0, 2)
        in_maps.append(
            {
                "xqT": xqT_,
                "xrT": xrT_,
                "yT": yT_,
                "mfT": mfT_,
                "ffT": ffT_,
                "wq": np.ascontiguousarray(wq_).astype(np.float16),
                "wk": np.ascontiguousarray(wk_).astype(np.float16),
                "wv": np.ascontiguousarray(wv_).astype(np.float16),
                "wp": np.ascontiguousarray(wp_).astype(np.float16),
                "bq": (bq[sl] * s).reshape(1, W).astype(np.float16),
                "bk": bk[sl].reshape(1, W).astype(np.float16),
                "bvs": bv.sum(0)[sl].reshape(1, W).astype(np.float16),
                "ones_r": np.ones((1, 512), np.float16),
                "ones_c": np.ones((128, 1), np.float16),
                "ones_b": np.ones((128, 128), np.float32),
            }
        )
    return in_maps


def kernel(x_q, x_r, y, mask, dist, Wq, bq, Wk, bk, Wv, bv, Wp, bp):
    inputs = dict(
        x_q=x_q, x_r=x_r, y=y, mask=mask, dist=dist,
        Wq=Wq, bq=bq, Wk=Wk, bk=bk, Wv=Wv, bv=bv, Wp=Wp, bp=bp,
    )
    in_maps = make_in_maps(inputs)
    nc = _get_nc()
    last = None
    for _ in range(3):
        try:
            res = run_bass_kernel_spmd(nc, in_maps, list(range(NCORES)))
            break
        except Exception as e:  # transient NRT device errors: retry
            last = e
    else:
        raise last

    out = np.zeros((B, T1, C), np.float32)
    for core in range(NCORES):
        b = core // HG
        z = res.results[core]["zpart"]  # [tc, er, 128, 512]
        out[b] += z.transpose(0, 2, 1, 3).reshape(T1, C)
    out += np.asarray(bp, np.float32)[None, None, :]
    return out



# revision 50
# speedup vs baseline: 1.4187x; 1.4187x over previous
"""Trainium2 Bass kernel for nn_CrossAttention (masked+distance-modulated cross attention).

Sharding: 8 cores = batch(2) x head-groups(4). Each core computes 4 of 16 heads
for one batch element, producing a partial output projection; partials are
summed on host (Wp is row-sharded by head).

Per-core dataflow (tuned for the instruction cost model where a matmul costs
output_free_size rows regardless of M/K):
  K^T = Wk^T x_r^T            [256, 2048] d-major, streamed in 512-col quarters
  Q^T = Wq^T x_q^T            per-t1-range, interleaved into the attention loop
  V   = sum_i y_i^T' Wv_i+bv  [2048s, 256] s-major, JIT per s-chunk in range 0
  per (t1-range 512, s-chunk 128, head-pair):
    S^T pair = K^T(chunk)^T Q^T(range)    [128s, 2x512t] one PSUM pair tile
    E = exp(S) (one ACT op over the pair)
    P1 = E*(mask*dmod), P2 = E*mask       (DVE/GPSIMD)
    per head, t-chunk 128: O[t,d] += P1(chunk)^T V  (N=64 -> cheap)
                           den[t] += P2(chunk)^T 1  (N=1  -> ~free)
  normalize: recip(den) -> per-partition scalar mul -> oN [128t, 128d-pair]
  transpose oN -> oT[d, t] via DMA-transpose (no engine time)
  Z = oT^T Wp per t-chunk   [128t, 512c] f16 partials -> host sums + bp
"""

import sys

sys.path.insert(0, "/opt/trn_rl_repo")

import numpy as np

import concourse.bass as bass
import concourse.mybir as mybir
import concourse.tile as tile
from concourse import bacc
from concourse.bass import ts
from concourse.bass_utils import run_bass_kernel_spmd

F32 = mybir.dt.float32
F16 = mybir.dt.float16
Exp = mybir.ActivationFunctionType.Exp

# problem dims (hardcoded per contract)
B, T1, T2, C, NH, NI = 2, 2048, 2048, 1024, 16, 3
GAMMA = 0.5
NCORES = 8
HG = 4            # head groups (cores per batch)
HPG = NH // HG    # heads per group = 4
HD = C // NH      # 64
W = HPG * HD      # local width = 256

CK_N = C // 128   # 8 contraction chunks for projections
JK_N = W // 128   # 2 output-chunks for Q^T/K^T
TR_N = T1 // 512  # 4 t1 ranges
SC_N = T2 // 128  # 16 s chunks
ER_N = C // 512   # 2 column ranges for Z


def build_core_program():
    nc = bacc.Bacc(None, target_bir_lowering=False, debug=False)

    xq = nc.declare_dram_parameter("xqT", [TR_N, 128, CK_N, 512], F16, isOutput=False)
    xr = nc.declare_dram_parameter("xrT", [TR_N, 128, CK_N, 512], F16, isOutput=False)
    yt = nc.declare_dram_parameter("yT", [SC_N, 128, NI, CK_N, 128], F16, isOutput=False)
    mfT = nc.declare_dram_parameter("mfT", [TR_N, 128, SC_N, 512], F16, isOutput=False)
    ffT = nc.declare_dram_parameter("ffT", [TR_N, 128, SC_N, 512], F16, isOutput=False)
    wq = nc.declare_dram_parameter("wq", [128, CK_N, JK_N, 128], F16, isOutput=False)
    wk = nc.declare_dram_parameter("wk", [128, CK_N, JK_N, 128], F16, isOutput=False)
    wv = nc.declare_dram_parameter("wv", [128, NI, CK_N, W], F16, isOutput=False)
    wp = nc.declare_dram_parameter("wp", [128, JK_N, C], F16, isOutput=False)
    bq2 = nc.declare_dram_parameter("bq2", [128, JK_N], F32, isOutput=False)
    bk2 = nc.declare_dram_parameter("bk2", [128, JK_N], F32, isOutput=False)
    bvs = nc.declare_dram_parameter("bvs", [1, W], F16, isOutput=False)
    ones_r = nc.declare_dram_parameter("ones_r", [1, 128], F16, isOutput=False)
    ones_c = nc.declare_dram_parameter("ones_c", [128, 1], F16, isOutput=False)
    zp = nc.declare_dram_parameter("zpart", [TR_N * 4, ER_N, 128, 512], F16, isOutput=True)

    with tile.TileContext(nc) as tc_:
        with tc_.tile_pool(name="persist", bufs=1) as pers:
            wq_sb = pers.tile([128, CK_N, JK_N, 128], F16, tag="wq")
            wk_sb = pers.tile([128, CK_N, JK_N, 128], F16, tag="wk")
            wv_sb = pers.tile([128, NI, CK_N, W], F16, tag="wv")
            wp_sb = pers.tile([128, JK_N, C], F16, tag="wp")
            bq2_sb = pers.tile([128, JK_N], F32, tag="bq2")
            bk2_sb = pers.tile([128, JK_N], F32, tag="bk2")
            bvs_sb = pers.tile([1, W], F16, tag="bvs")
            ones_r_sb = pers.tile([1, 128], F16, tag="ones_r")
            ones_c_sb = pers.tile([128, 1], F16, tag="ones_c")

            # FIFO DMA queue: smallest/earliest-needed first
            nc.sync.dma_start(wk_sb[:], wk[:])
            nc.sync.dma_start(bk2_sb[:], bk2[:])

            qT = pers.tile([128, JK_N, T1], F16, tag="qT")
            kT = pers.tile([128, JK_N, T2], F16, tag="kT")
            v_sb = pers.tile([128, SC_N, W], F16, tag="v")
            oT = pers.tile([128, JK_N, T1], F16, tag="oT")

            def proj_mms(pp_tile, w_sb, x_tile, jk):
                for ck in range(CK_N):
                    nc.tensor.matmul(
                        pp_tile[:],
                        w_sb[:, ck, jk, :],
                        x_tile[:, ck, :],
                        start=(ck == 0),
                        stop=(ck == CK_N - 1),
                    )

            # ---- phase A: K^T (quarters) + Q^T for tr0, tr1 ----
            with (
                tc_.tile_pool(name="pp", bufs=2, space="PSUM") as pp,
                tc_.tile_pool(name="xpool", bufs=2) as xpool,
            ):
                for q in range(4):
                    xt = xpool.tile([128, CK_N, 512], F16, tag="xr", name=f"xr{q}")
                    if q == 0:
                        # per-ck loads so the first matmul starts ~0.4us in
                        for ck in range(CK_N):
                            nc.sync.dma_start(xt[:, ck, :], xr[0, :, ck, :])
                    else:
                        nc.sync.dma_start(xt[:], xr[q])
                    if q == 3:
                        nc.sync.dma_start(wq_sb[:], wq[:])
                        nc.sync.dma_start(bq2_sb[:], bq2[:])
                        nc.sync.dma_start(ones_r_sb[:], ones_r[:])
                        nc.sync.dma_start(ones_c_sb[:], ones_c[:])
                    for jk in range(JK_N):
                        pst = pp.tile([128, 512], F32, tag="p")
                        proj_mms(pst, wk_sb, xt, jk)
                        nc.scalar.add(kT[:, jk, ts(q, 512)], pst[:], add=bk2_sb[:, jk : jk + 1])
                xt = xpool.tile([128, CK_N, 512], F16, tag="xq", name="xq0")
                nc.sync.dma_start(xt[:], xq[0])
                for jk in range(JK_N):
                    pst = pp.tile([128, 512], F32, tag="p")
                    proj_mms(pst, wq_sb, xt, jk)
                    nc.scalar.add(qT[:, jk, 0:512], pst[:], add=bq2_sb[:, jk : jk + 1])

            # remaining weights + first mask slabs
            nc.sync.dma_start(wv_sb[:], wv[:])
            nc.sync.dma_start(bvs_sb[:], bvs[:])
            nc.sync.dma_start(wp_sb[:], wp[:])

            mpool_cm = tc_.tile_pool(name="mpool", bufs=2)
            mpool = mpool_cm.__enter__()
            mslabs = {}

            def alloc_mslab(tr):
                mf_all = mpool.tile([128, SC_N, 512], F16, tag="mfall", name=f"mfall{tr}")
                ff_all = mpool.tile([128, SC_N, 512], F16, tag="ffall", name=f"ffall{tr}")
                mslabs[tr] = (mf_all, ff_all)
                return mslabs[tr]

            def load_mslab_half(tr, h):
                """Half-slab loads: keeps mask DMAs from head-of-line blocking y."""
                slab = mslabs.get(tr) or alloc_mslab(tr)
                mf_all, ff_all = slab
                sl = slice(h * 8, (h + 1) * 8)
                nc.sync.dma_start(mf_all[:, sl, :], mfT[tr, :, sl, :])
                nc.sync.dma_start(ff_all[:, sl, :], ffT[tr, :, sl, :])

            load_mslab_half(0, 0)
            load_mslab_half(0, 1)

            # ---- phase B ----
            with (
                tc_.tile_pool(name="spool", bufs=2, space="PSUM") as spool,
                tc_.tile_pool(name="popool", bufs=2, space="PSUM") as popool,
                tc_.tile_pool(name="ddpool", bufs=1, space="PSUM") as ddpool,
                tc_.tile_pool(name="auxp", bufs=1, space="PSUM") as auxp,
                tc_.tile_pool(name="ypool", bufs=2) as ypool,
                tc_.tile_pool(name="attn", bufs=6) as attn,
                tc_.tile_pool(name="xqpool", bufs=1) as xqpool,
                tc_.tile_pool(name="onpool", bufs=8) as onpool,
                tc_.tile_pool(name="rpool", bufs=2) as rpool,
                tc_.tile_pool(name="pcache", bufs=2) as pcache,
                tc_.tile_pool(name="zoutp", bufs=6) as zoutp,
            ):
                def emit_z_tile(ztr, tcl, er):
                    tcc = ztr * 4 + tcl
                    psz = auxp.tile([128, 512], F32, tag="aux", name=f"z{tcc}_{er}")
                    for jk in range(JK_N):
                        nc.tensor.matmul(
                            psz[:],
                            oT[:, jk, ts(tcc, 128)],
                            wp_sb[:, jk, ts(er, 512)],
                            start=(jk == 0),
                            stop=(jk == JK_N - 1),
                        )
                    z_t = zoutp.tile([128, 512], F16, tag="zt")
                    if er == 0:
                        nc.vector.tensor_copy(z_t[:], psz[:])
                    else:
                        nc.scalar.copy(z_t[:], psz[:])
                    nc.sync.dma_start(zp[tcc, er], z_t[:])

                q_pend = {}

                def emit_q_chain(qtr, jk, xq_t):
                    pq = auxp.tile([128, 512], F32, tag="aux", name=f"q{qtr}_{jk}")
                    proj_mms(pq, wq_sb, xq_t, jk)
                    q_pend[jk] = (qtr, pq)

                def emit_q_copy(jk):
                    qtr_, pq = q_pend.pop(jk)
                    nc.vector.tensor_scalar_add(
                        qT[:, jk, ts(qtr_, 512)], pq[:], bq2_sb[:, jk : jk + 1]
                    )

                pend_av = []  # AV emission 3 units behind (covers exp+mul latency)
                pend_den = []  # den emission 8 units behind (covers Pool latency)

                def emit_av(st):
                    tr, hp, sc, p1, p2, po_t, dd_t = st
                    for i in range(2):
                        h = 2 * hp + i
                        for tcl in range(4):
                            nc.tensor.matmul(
                                po_t[:, (4 * i + tcl) * 64 : (4 * i + tcl + 1) * 64],
                                p1[:, i * 512 + tcl * 128 : i * 512 + (tcl + 1) * 128],
                                v_sb[:, sc, ts(h, 64)],
                                # start=True zeroes the WHOLE tile: only the
                                # first-emitted chain per po tile may use it
                                # (sibling open chains would be wiped).
                                start=(sc == 0 and i == 0 and tcl == 0),
                                stop=(sc == SC_N - 1),
                                skip_group_check=True,
                            )

                def emit_den(st):
                    tr, hp, sc, p1, p2, po_t, dd_t = st
                    for i in range(2):
                        for tcl in range(4):
                            nc.tensor.matmul(
                                dd_t[:, hp * 8 + 4 * i + tcl : hp * 8 + 4 * i + tcl + 1],
                                p2[:, i * 512 + tcl * 128 : i * 512 + (tcl + 1) * 128],
                                ones_c_sb[:],
                                start=(sc == 0 and hp == 0 and i == 0 and tcl == 0),
                                stop=(sc == SC_N - 1),
                                skip_group_check=True,
                            )

                def emit_norm_unit(ntr, hp, tcl, po_t, rec):
                    """2 scalar muls + 1 DMA transpose for one (hp, t-chunk).
                    po_t may be the PSUM tile (tail) or the f16 SBUF cache
                    (deferred path, safe against PSUM buffer recycling)."""
                    oN = onpool.tile([128, 128], F16, tag="oN")
                    for i in range(2):
                        nc.vector.tensor_scalar_mul(
                            oN[:, i * 64 : (i + 1) * 64],
                            po_t[:, (4 * i + tcl) * 64 : (4 * i + tcl + 1) * 64],
                            rec[:, 4 * i + tcl : 4 * i + tcl + 1],
                        )
                    nc.sync.dma_start_transpose(oT[:, hp, ts(4 * ntr + tcl, 128)], oN[:])

                xq_next = None
                prev_norm = None  # (tr, po_tiles, recs) deferred into next range
                unit = 0  # global (sc, hp) unit counter for engine splits
                ytiles = {}

                def issue_y(sc):
                    ysc = ypool.tile([128, NI, CK_N, 128], F16, tag="y")
                    nc.sync.dma_start(ysc[:], yt[sc])
                    ytiles[sc] = ysc

                def emit_v_chunk(sc):
                    ysc = ytiles.pop(sc)
                    pvt = auxp.tile([128, 512], F32, tag="aux", name=f"v{sc}")
                    for i in range(NI):
                        for ck in range(CK_N):
                            nc.tensor.matmul(
                                pvt[:, 0:W],
                                ysc[:, i, ck, :],
                                wv_sb[:, i, ck, :],
                                start=(i == 0 and ck == 0),
                                stop=False,
                            )
                    nc.tensor.matmul(
                        pvt[:, 0:W],
                        ones_r_sb[0:1, :],
                        bvs_sb[0:1, :],
                        start=False,
                        stop=True,
                    )
                    nc.vector.tensor_copy(v_sb[:, sc, :], pvt[:, 0:W])

                for tr in range(TR_N):
                    mf_all, ff_all = mslabs.pop(tr)
                    qtr = tr + 1  # Q(tr0) done in phase A; tr k projects Q(k+1)
                    if tr == 0:
                        xq_next = xqpool.tile([128, CK_N, 512], F16, tag="xq", name="xqs1")
                        nc.sync.dma_start(xq_next[:], xq[1])
                    po_hp = [
                        popool.tile([128, 512], F32, tag="po", name=f"po{tr}_{hp}")
                        for hp in range(2)
                    ]
                    dd_t = ddpool.tile([128, 16], F32, tag="dd", name=f"dd{tr}")

                    if tr == 0:
                        issue_y(0)
                        emit_v_chunk(0)
                    for sc in range(SC_N):
                        for hp in range(2):
                            s_t = spool.tile([128, 1024], F32, tag="s")
                            for i in range(2):
                                h = 2 * hp + i
                                off = 64 * (h % 2)
                                hk = h // 2
                                nc.tensor.matmul(
                                    s_t[:, i * 512 : (i + 1) * 512],
                                    kT[off : off + 64, hk, ts(sc, 128)],
                                    qT[off : off + 64, hk, ts(tr, 512)],
                                    start=True,
                                    stop=True,
                                )
                            e_t = attn.tile([128, 1024], F16, tag="e", bufs=8)
                            nc.scalar.activation(e_t[:], s_t[:], Exp)
                            p1 = attn.tile([128, 1024], F16, tag="p1", bufs=8)
                            p2 = attn.tile([128, 1024], F16, tag="p2", bufs=10)
                            # P1 feeds AV (critical) -> always DVE. Pool takes
                            # ~0.75 P2 halves per unit (it must keep slack or
                            # its lag throttles exp via e-buffer recycling);
                            # den consumes with an 8-unit lag to hide Pool.
                            pool_pick = unit % 2
                            for i in range(2):
                                sl = slice(i * 512, (i + 1) * 512)
                                nc.vector.tensor_mul(p1[:, sl], e_t[:, sl], ff_all[:, sc, :])
                                if i == pool_pick:
                                    nc.gpsimd.tensor_mul(p2[:, sl], e_t[:, sl], mf_all[:, sc, :])
                                else:
                                    nc.vector.tensor_mul(p2[:, sl], e_t[:, sl], mf_all[:, sc, :])
                            unit += 1
                            st = (tr, hp, sc, p1, p2, po_hp[hp], dd_t)
                            if len(pend_av) >= 3:
                                emit_av(pend_av.pop(0))
                            pend_av.append(st)
                            if len(pend_den) >= 8:
                                emit_den(pend_den.pop(0))
                            pend_den.append(st)
                        # ---- interleaved extras (after S-pairs so ACT isn't starved) ----
                        if tr == 0:
                            if sc + 1 < SC_N:
                                issue_y(sc + 1)
                                emit_v_chunk(sc + 1)
                        if prev_norm is not None and sc < 4:
                            ptr_, ppo, precs = prev_norm
                            for k in (2 * sc, 2 * sc + 1):
                                emit_norm_unit(ptr_, k // 4, k % 4, ppo[k // 4], precs[k // 4])
                        if tr > 0 and 6 <= sc < 14:
                            idx = sc - 6
                            emit_z_tile(tr - 1, idx // 2, idx % 2)
                        if qtr < TR_N:
                            if sc == 2:
                                emit_q_chain(qtr, 0, xq_next)
                            elif sc == 4:
                                emit_q_copy(0)
                            elif sc == 5:
                                emit_q_chain(qtr, 1, xq_next)
                            elif sc == 7:
                                emit_q_copy(1)
                        if tr + 1 < TR_N:
                            if sc == 6:
                                load_mslab_half(tr + 1, 0)
                            elif sc == 8:
                                load_mslab_half(tr + 1, 1)
                            elif sc == 12 and tr + 2 < TR_N:
                                # prefetch next range's xq slab
                                xq_next = xqpool.tile(
                                    [128, CK_N, 512], F16, tag="xq", name=f"xqs{tr + 2}"
                                )
                                nc.sync.dma_start(xq_next[:], xq[tr + 2])
                    # flush pending AVs/dens, then reciprocals (frees dd).
                    # In the last range, flush hp0's dens first so its norm
                    # and transposes start before hp1's AVs finish.
                    last = tr == TR_N - 1
                    while pend_av:
                        emit_av(pend_av.pop(0))
                    recs = [None, None]
                    for hp in range(2):
                        rest = []
                        for st in pend_den:
                            if last and st[1] != hp:
                                rest.append(st)
                            else:
                                emit_den(st)
                        pend_den = rest
                        rec = rpool.tile([128, 8], F32, tag="rec")
                        nc.vector.reciprocal(rec[:], dd_t[:, hp * 8 : hp * 8 + 8])
                        recs[hp] = rec
                        if last:
                            for tcl in range(4):
                                emit_norm_unit(tr, hp, tcl, po_hp[hp], rec)
                        if not last:
                            break
                    if not last:
                        rec = rpool.tile([128, 8], F32, tag="rec")
                        nc.vector.reciprocal(rec[:], dd_t[:, 8:16])
                        recs[1] = rec
                    if not last:
                        # snapshot po to SBUF f16 before the next range's AVs
                        # recycle the PSUM buffers (deferred norm reads this)
                        pc = []
                        for hp in range(2):
                            c = pcache.tile([128, 512], F16, tag="pc", name=f"pc{tr}_{hp}")
                            nc.vector.tensor_copy(c[:], po_hp[hp][:])
                            pc.append(c)
                        prev_norm = (tr, pc, recs)
                    else:
                        prev_norm = (tr, po_hp, recs)
                # ---- tail: Z via the freed s-pool (2-deep) ----
                ltr, lpo, lrecs = prev_norm
                for tcl in range(4):
                    tcc = ltr * 4 + tcl
                    psz = spool.tile([128, 1024], F32, tag="s", name=f"zt{tcl}")
                    for er in range(ER_N):
                        for jk in range(JK_N):
                            nc.tensor.matmul(
                                psz[:, er * 512 : (er + 1) * 512],
                                oT[:, jk, ts(tcc, 128)],
                                wp_sb[:, jk, ts(er, 512)],
                                start=(jk == 0),
                                stop=(jk == JK_N - 1),
                            )
                    for er in range(ER_N):
                        z_t = zoutp.tile([128, 512], F16, tag="zt")
                        if er == 0:
                            nc.vector.tensor_copy(z_t[:], psz[:, 0:512])
                        else:
                            nc.scalar.copy(z_t[:], psz[:, 512:1024])
                        nc.sync.dma_start(zp[tcc, er], z_t[:])

            mpool_cm.__exit__(None, None, None)

    nc.compile()
    return nc


_NC = None


def _get_nc():
    global _NC
    if _NC is None:
        _NC = build_core_program()
    return _NC


def make_in_maps(inputs):
    x_q = np.asarray(inputs["x_q"], np.float32)
    x_r = np.asarray(inputs["x_r"], np.float32)
    y = np.asarray(inputs["y"], np.float32)
    mask = np.asarray(inputs["mask"])
    dist = np.asarray(inputs["dist"], np.float32)
    Wq, bq, Wk, bk, Wv, bv, Wp, bp = (
        np.asarray(inputs[k], np.float32)
        for k in ("Wq", "bq", "Wk", "bk", "Wv", "bv", "Wp", "bp")
    )

    s = np.float32(1.0 / np.sqrt(HD))

    per_batch = []
    for b in range(B):
        maskf = (mask[b, 0] != 0).astype(np.float32)  # [T1, T2]
        dmod = np.exp(-np.square(dist[b, 0] / GAMMA)).astype(np.float32)

        # [s, t] tiled as [tr, 128(s-part), sc, 512(t)]
        def tile_st(a):
            return np.ascontiguousarray(
                a.T.reshape(SC_N, 128, TR_N, 512).transpose(2, 1, 0, 3)
            ).astype(np.float16)

        mfT_ = tile_st(maskf)
        ffT_ = tile_st(maskf * dmod)

        # x^T [c, t] -> [tr/quarter 4, 128, ck, 512]
        def tile_x(a):
            aT = a.T.reshape(CK_N, 128, 4, 512)
            return np.ascontiguousarray(aT.transpose(2, 1, 0, 3)).astype(np.float16)

        xqT_ = tile_x(x_q[b])
        xrT_ = tile_x(x_r[b])
        # y[:, b] [NI, T2, C] -> [sc, 128p(c), NI, ck, 128(s)]
        yb = y[:, b].reshape(NI, SC_N, 128, CK_N, 128)  # i, sc, sl, ck, p
        yT_ = np.ascontiguousarray(yb.transpose(1, 4, 0, 3, 2)).astype(np.float16)
        per_batch.append((xqT_, xrT_, yT_, mfT_, ffT_))

    in_maps = []
    for core in range(NCORES):
        b, hg = divmod(core, HG)
        sl = slice(hg * W, (hg + 1) * W)
        xqT_, xrT_, yT_, mfT_, ffT_ = per_batch[b]
        wq_ = (Wq[:, sl] * s).reshape(CK_N, 128, JK_N, 128).transpose(1, 0, 2, 3)
        wk_ = Wk[:, sl].reshape(CK_N, 128, JK_N, 128).transpose(1, 0, 2, 3)
        wv_ = Wv[:, :, sl].reshape(NI, CK_N, 128, W).transpose(2, 0, 1, 3)
        wp_ = Wp[sl, :].reshape(JK_N, 128, C).transpose(1, 0, 2)
        in_maps.append(
            {
                "xqT": xqT_,
                "xrT": xrT_,
                "yT": yT_,
                "mfT": mfT_,
                "ffT": ffT_,
                "wq": np.ascontiguousarray(wq_).astype(np.float16),
                "wk": np.ascontiguousarray(wk_).astype(np.float16),
                "wv": np.ascontiguousarray(wv_).astype(np.float16),
                "wp": np.ascontiguousarray(wp_).astype(np.float16),
                "bq2": np.ascontiguousarray((bq[sl] * s).reshape(JK_N, 128).T).astype(np.float32),
                "bk2": np.ascontiguousarray(bk[sl].reshape(JK_N, 128).T).astype(np.float32),
                "bvs": bv.sum(0)[sl].reshape(1, W).astype(np.float16),
                "ones_r": np.ones((1, 128), np.float16),
                "ones_c": np.ones((128, 1), np.float16),
            }
        )
    return in_maps


def kernel(x_q, x_r, y, mask, dist, Wq, bq, Wk, bk, Wv, bv, Wp, bp):
    inputs = dict(
        x_q=x_q, x_r=x_r, y=y, mask=mask, dist=dist,
        Wq=Wq, bq=bq, Wk=Wk, bk=bk, Wv=Wv, bv=bv, Wp=Wp, bp=bp,
    )
    in_maps = make_in_maps(inputs)
    nc = _get_nc()
    last = None
    for _ in range(3):
        try:
            res = run_bass_kernel_spmd(nc, in_maps, list(range(NCORES)))
            break
        except Exception as e:  # transient NRT device errors: retry
            last = e
    else:
        raise last

    out = np.zeros((B, T1, C), np.float32)
    for core in range(NCORES):
        b = core // HG
        z = res.results[core]["zpart"]  # [tcc, er, 128, 512]
        out[b] += z.astype(np.float32).transpose(0, 2, 1, 3).reshape(T1, C)
    out += np.asarray(bp, np.float32)[None, None, :]
    return out


# revision 53
# speedup vs baseline: 1.4233x; 1.0033x over previous
"""Trainium2 Bass kernel for nn_CrossAttention (masked+distance-modulated cross attention).

Sharding: 8 cores = batch(2) x head-groups(4). Each core computes 4 of 16 heads
for one batch element, producing a partial output projection; partials are
summed on host (Wp is row-sharded by head).

Per-core dataflow (tuned for the instruction cost model where a matmul costs
output_free_size rows regardless of M/K):
  K^T = Wk^T x_r^T            [256, 2048] d-major, streamed in 512-col quarters
  Q^T = Wq^T x_q^T            per-t1-range, interleaved into the attention loop
  V   = sum_i y_i^T' Wv_i+bv  [2048s, 256] s-major, JIT per s-chunk in range 0
  per (t1-range 512, s-chunk 128, head-pair):
    S^T pair = K^T(chunk)^T Q^T(range)    [128s, 2x512t] one PSUM pair tile
    E = exp(S) (one ACT op over the pair)
    P1 = E*(mask*dmod), P2 = E*mask       (DVE/GPSIMD)
    per head, t-chunk 128: O[t,d] += P1(chunk)^T V  (N=64 -> cheap)
                           den[t] += P2(chunk)^T 1  (N=1  -> ~free)
  normalize: recip(den) -> per-partition scalar mul -> oN [128t, 128d-pair]
  transpose oN -> oT[d, t] via DMA-transpose (no engine time)
  Z = oT^T Wp per t-chunk   [128t, 512c] f16 partials -> host sums + bp
"""

import sys

sys.path.insert(0, "/opt/trn_rl_repo")

import numpy as np

import concourse.bass as bass
import concourse.mybir as mybir
import concourse.tile as tile
from concourse import bacc
from concourse.bass import ts
from concourse.bass_utils import run_bass_kernel_spmd

F32 = mybir.dt.float32
F16 = mybir.dt.float16
Exp = mybir.ActivationFunctionType.Exp

# problem dims (hardcoded per contract)
B, T1, T2, C, NH, NI = 2, 2048, 2048, 1024, 16, 3
GAMMA = 0.5
NCORES = 8
HG = 4            # head groups (cores per batch)
HPG = NH // HG    # heads per group = 4
HD = C // NH      # 64
W = HPG * HD      # local width = 256

CK_N = C // 128   # 8 contraction chunks for projections
JK_N = W // 128   # 2 output-chunks for Q^T/K^T
TR_N = T1 // 512  # 4 t1 ranges
SC_N = T2 // 128  # 16 s chunks
ER_N = C // 512   # 2 column ranges for Z


def build_core_program():
    nc = bacc.Bacc(None, target_bir_lowering=False, debug=False)

    xq = nc.declare_dram_parameter("xqT", [TR_N, 128, CK_N, 512], F16, isOutput=False)
    xr = nc.declare_dram_parameter("xrT", [TR_N, 128, CK_N, 512], F16, isOutput=False)
    yt = nc.declare_dram_parameter("yT", [SC_N, 128, NI, CK_N, 128], F16, isOutput=False)
    mfT = nc.declare_dram_parameter("mfT", [TR_N, 128, SC_N, 512], F16, isOutput=False)
    ffT = nc.declare_dram_parameter("ffT", [TR_N, 128, SC_N, 512], F16, isOutput=False)
    wq = nc.declare_dram_parameter("wq", [128, CK_N, JK_N, 128], F16, isOutput=False)
    wk = nc.declare_dram_parameter("wk", [128, CK_N, JK_N, 128], F16, isOutput=False)
    wv = nc.declare_dram_parameter("wv", [128, NI, CK_N, W], F16, isOutput=False)
    wp = nc.declare_dram_parameter("wp", [128, JK_N, C], F16, isOutput=False)
    bq2 = nc.declare_dram_parameter("bq2", [128, JK_N], F32, isOutput=False)
    bk2 = nc.declare_dram_parameter("bk2", [128, JK_N], F32, isOutput=False)
    bvs = nc.declare_dram_parameter("bvs", [1, W], F16, isOutput=False)
    ones_r = nc.declare_dram_parameter("ones_r", [1, 128], F16, isOutput=False)
    ones_c = nc.declare_dram_parameter("ones_c", [128, 1], F16, isOutput=False)
    zp = nc.declare_dram_parameter("zpart", [TR_N * 4, ER_N, 128, 512], F16, isOutput=True)

    with tile.TileContext(nc) as tc_:
        with tc_.tile_pool(name="persist", bufs=1) as pers:
            wq_sb = pers.tile([128, CK_N, JK_N, 128], F16, tag="wq")
            wk_sb = pers.tile([128, CK_N, JK_N, 128], F16, tag="wk")
            wv_sb = pers.tile([128, NI, CK_N, W], F16, tag="wv")
            wp_sb = pers.tile([128, JK_N, C], F16, tag="wp")
            bq2_sb = pers.tile([128, JK_N], F32, tag="bq2")
            bk2_sb = pers.tile([128, JK_N], F32, tag="bk2")
            bvs_sb = pers.tile([1, W], F16, tag="bvs")
            ones_r_sb = pers.tile([1, 128], F16, tag="ones_r")
            ones_c_sb = pers.tile([128, 1], F16, tag="ones_c")

            # FIFO DMA queue: smallest/earliest-needed first
            nc.sync.dma_start(wk_sb[:], wk[:])
            nc.sync.dma_start(bk2_sb[:], bk2[:])

            qT = pers.tile([128, JK_N, T1], F16, tag="qT")
            kT = pers.tile([128, JK_N, T2], F16, tag="kT")
            v_sb = pers.tile([128, SC_N, W], F16, tag="v")
            oT = pers.tile([128, JK_N, T1], F16, tag="oT")

            def proj_mms(pp_tile, w_sb, x_tile, jk):
                for ck in range(CK_N):
                    nc.tensor.matmul(
                        pp_tile[:],
                        w_sb[:, ck, jk, :],
                        x_tile[:, ck, :],
                        start=(ck == 0),
                        stop=(ck == CK_N - 1),
                    )

            # ---- phase A: K^T (quarters) + Q^T for tr0, tr1 ----
            with (
                tc_.tile_pool(name="pp", bufs=2, space="PSUM") as pp,
                tc_.tile_pool(name="xpool", bufs=2) as xpool,
            ):
                for q in range(4):
                    xt = xpool.tile([128, CK_N, 512], F16, tag="xr", name=f"xr{q}")
                    if q == 0:
                        # per-ck loads so the first matmul starts ~0.4us in
                        for ck in range(CK_N):
                            nc.sync.dma_start(xt[:, ck, :], xr[0, :, ck, :])
                    else:
                        nc.sync.dma_start(xt[:], xr[q])
                    if q == 3:
                        nc.sync.dma_start(wq_sb[:], wq[:])
                        nc.sync.dma_start(bq2_sb[:], bq2[:])
                        nc.sync.dma_start(ones_r_sb[:], ones_r[:])
                        nc.sync.dma_start(ones_c_sb[:], ones_c[:])
                    for jk in range(JK_N):
                        pst = pp.tile([128, 512], F32, tag="p")
                        proj_mms(pst, wk_sb, xt, jk)
                        nc.scalar.add(kT[:, jk, ts(q, 512)], pst[:], add=bk2_sb[:, jk : jk + 1])
                xt = xpool.tile([128, CK_N, 512], F16, tag="xq", name="xq0")
                nc.sync.dma_start(xt[:], xq[0])
                for jk in range(JK_N):
                    pst = pp.tile([128, 512], F32, tag="p")
                    proj_mms(pst, wq_sb, xt, jk)
                    nc.scalar.add(qT[:, jk, 0:512], pst[:], add=bq2_sb[:, jk : jk + 1])

            # remaining weights + first mask slabs
            nc.sync.dma_start(wv_sb[:], wv[:])
            nc.sync.dma_start(bvs_sb[:], bvs[:])
            nc.sync.dma_start(wp_sb[:], wp[:])

            mpool_cm = tc_.tile_pool(name="mpool", bufs=2)
            mpool = mpool_cm.__enter__()
            mslabs = {}

            def alloc_mslab(tr):
                mf_all = mpool.tile([128, SC_N, 512], F16, tag="mfall", name=f"mfall{tr}")
                ff_all = mpool.tile([128, SC_N, 512], F16, tag="ffall", name=f"ffall{tr}")
                mslabs[tr] = (mf_all, ff_all)
                return mslabs[tr]

            def load_mslab_half(tr, h):
                """Half-slab loads: keeps mask DMAs from head-of-line blocking y."""
                slab = mslabs.get(tr) or alloc_mslab(tr)
                mf_all, ff_all = slab
                sl = slice(h * 8, (h + 1) * 8)
                nc.sync.dma_start(mf_all[:, sl, :], mfT[tr, :, sl, :])
                nc.sync.dma_start(ff_all[:, sl, :], ffT[tr, :, sl, :])

            load_mslab_half(0, 0)
            load_mslab_half(0, 1)

            # ---- phase B ----
            with (
                tc_.tile_pool(name="spool", bufs=2, space="PSUM") as spool,
                tc_.tile_pool(name="popool", bufs=2, space="PSUM") as popool,
                tc_.tile_pool(name="ddpool", bufs=1, space="PSUM") as ddpool,
                tc_.tile_pool(name="auxp", bufs=1, space="PSUM") as auxp,
                tc_.tile_pool(name="ypool", bufs=2) as ypool,
                tc_.tile_pool(name="attn", bufs=6) as attn,
                tc_.tile_pool(name="xqpool", bufs=1) as xqpool,
                tc_.tile_pool(name="onpool", bufs=8) as onpool,
                tc_.tile_pool(name="rpool", bufs=2) as rpool,
                tc_.tile_pool(name="pcache", bufs=2) as pcache,
                tc_.tile_pool(name="zoutp", bufs=6) as zoutp,
            ):
                def emit_z_tile(ztr, tcl, er):
                    tcc = ztr * 4 + tcl
                    psz = auxp.tile([128, 512], F32, tag="aux", name=f"z{tcc}_{er}")
                    for jk in range(JK_N):
                        nc.tensor.matmul(
                            psz[:],
                            oT[:, jk, ts(tcc, 128)],
                            wp_sb[:, jk, ts(er, 512)],
                            start=(jk == 0),
                            stop=(jk == JK_N - 1),
                        )
                    z_t = zoutp.tile([128, 512], F16, tag="zt")
                    if er == 0:
                        nc.vector.tensor_copy(z_t[:], psz[:])
                    else:
                        nc.scalar.copy(z_t[:], psz[:])
                    nc.sync.dma_start(zp[tcc, er], z_t[:])

                q_pend = {}

                def emit_q_chain(qtr, jk, xq_t):
                    pq = auxp.tile([128, 512], F32, tag="aux", name=f"q{qtr}_{jk}")
                    proj_mms(pq, wq_sb, xq_t, jk)
                    q_pend[jk] = (qtr, pq)

                def emit_q_copy(jk):
                    qtr_, pq = q_pend.pop(jk)
                    nc.vector.tensor_scalar_add(
                        qT[:, jk, ts(qtr_, 512)], pq[:], bq2_sb[:, jk : jk + 1]
                    )

                pend_av = []  # AV emission 3 units behind (covers exp+mul latency)
                pend_den = []  # den emission 8 units behind (covers Pool latency)

                def emit_av(st):
                    tr, hp, sc, p1, p2, po_t, dd_t = st
                    for i in range(2):
                        h = 2 * hp + i
                        for tcl in range(4):
                            nc.tensor.matmul(
                                po_t[:, (4 * i + tcl) * 64 : (4 * i + tcl + 1) * 64],
                                p1[:, i * 512 + tcl * 128 : i * 512 + (tcl + 1) * 128],
                                v_sb[:, sc, ts(h, 64)],
                                # start=True zeroes the WHOLE tile: only the
                                # first-emitted chain per po tile may use it
                                # (sibling open chains would be wiped).
                                start=(sc == 0 and i == 0 and tcl == 0),
                                stop=(sc == SC_N - 1),
                                skip_group_check=True,
                            )

                def emit_den(st):
                    tr, hp, sc, p1, p2, po_t, dd_t = st
                    for i in range(2):
                        for tcl in range(4):
                            nc.tensor.matmul(
                                dd_t[:, hp * 8 + 4 * i + tcl : hp * 8 + 4 * i + tcl + 1],
                                p2[:, i * 512 + tcl * 128 : i * 512 + (tcl + 1) * 128],
                                ones_c_sb[:],
                                start=(sc == 0 and hp == 0 and i == 0 and tcl == 0),
                                stop=(sc == SC_N - 1),
                                skip_group_check=True,
                            )

                def emit_norm_unit(ntr, hp, tcl, po_t, rec):
                    """2 scalar muls + 1 DMA transpose for one (hp, t-chunk).
                    po_t may be the PSUM tile (tail) or the f16 SBUF cache
                    (deferred path, safe against PSUM buffer recycling)."""
                    oN = onpool.tile([128, 128], F16, tag="oN")
                    for i in range(2):
                        nc.vector.tensor_scalar_mul(
                            oN[:, i * 64 : (i + 1) * 64],
                            po_t[:, (4 * i + tcl) * 64 : (4 * i + tcl + 1) * 64],
                            rec[:, 4 * i + tcl : 4 * i + tcl + 1],
                        )
                    nc.sync.dma_start_transpose(oT[:, hp, ts(4 * ntr + tcl, 128)], oN[:])

                xq_next = None
                prev_norm = None  # (tr, po_tiles, recs) deferred into next range
                unit = 0  # global (sc, hp) unit counter for engine splits
                ytiles = {}

                def issue_y(sc):
                    ysc = ypool.tile([128, NI, CK_N, 128], F16, tag="y")
                    nc.sync.dma_start(ysc[:], yt[sc])
                    ytiles[sc] = ysc

                def emit_v_chunk(sc):
                    ysc = ytiles.pop(sc)
                    pvt = auxp.tile([128, 512], F32, tag="aux", name=f"v{sc}")
                    for i in range(NI):
                        for ck in range(CK_N):
                            nc.tensor.matmul(
                                pvt[:, 0:W],
                                ysc[:, i, ck, :],
                                wv_sb[:, i, ck, :],
                                start=(i == 0 and ck == 0),
                                stop=False,
                            )
                    nc.tensor.matmul(
                        pvt[:, 0:W],
                        ones_r_sb[0:1, :],
                        bvs_sb[0:1, :],
                        start=False,
                        stop=True,
                    )
                    nc.vector.tensor_copy(v_sb[:, sc, :], pvt[:, 0:W])

                for tr in range(TR_N):
                    mf_all, ff_all = mslabs.pop(tr)
                    qtr = tr + 1  # Q(tr0) done in phase A; tr k projects Q(k+1)
                    if tr == 0:
                        xq_next = xqpool.tile([128, CK_N, 512], F16, tag="xq", name="xqs1")
                        nc.sync.dma_start(xq_next[:], xq[1])
                    po_hp = [
                        popool.tile([128, 512], F32, tag="po", name=f"po{tr}_{hp}")
                        for hp in range(2)
                    ]
                    dd_t = ddpool.tile([128, 16], F32, tag="dd", name=f"dd{tr}")

                    if tr == 0:
                        issue_y(0)
                        emit_v_chunk(0)
                    for sc in range(SC_N):
                        for hp in range(2):
                            s_t = spool.tile([128, 1024], F32, tag="s")
                            for i in range(2):
                                h = 2 * hp + i
                                off = 64 * (h % 2)
                                hk = h // 2
                                nc.tensor.matmul(
                                    s_t[:, i * 512 : (i + 1) * 512],
                                    kT[off : off + 64, hk, ts(sc, 128)],
                                    qT[off : off + 64, hk, ts(tr, 512)],
                                    start=True,
                                    stop=True,
                                )
                            e_t = attn.tile([128, 1024], F16, tag="e", bufs=8)
                            nc.scalar.activation(e_t[:], s_t[:], Exp)
                            p1 = attn.tile([128, 1024], F16, tag="p1", bufs=8)
                            p2 = attn.tile([128, 1024], F16, tag="p2", bufs=10)
                            # P1 feeds AV (critical) -> always DVE. Pool takes
                            # ~0.75 P2 halves per unit (it must keep slack or
                            # its lag throttles exp via e-buffer recycling);
                            # den consumes with an 8-unit lag to hide Pool.
                            pool_pick = unit % 2
                            for i in range(2):
                                sl = slice(i * 512, (i + 1) * 512)
                                nc.vector.tensor_mul(p1[:, sl], e_t[:, sl], ff_all[:, sc, :])
                                if i == pool_pick:
                                    nc.gpsimd.tensor_mul(p2[:, sl], e_t[:, sl], mf_all[:, sc, :])
                                else:
                                    nc.vector.tensor_mul(p2[:, sl], e_t[:, sl], mf_all[:, sc, :])
                            unit += 1
                            st = (tr, hp, sc, p1, p2, po_hp[hp], dd_t)
                            if len(pend_av) >= 4:
                                emit_av(pend_av.pop(0))
                            pend_av.append(st)
                            if len(pend_den) >= 8:
                                emit_den(pend_den.pop(0))
                            pend_den.append(st)
                        # ---- interleaved extras (after S-pairs so ACT isn't starved) ----
                        if tr == 0:
                            if sc + 1 < SC_N:
                                issue_y(sc + 1)
                                emit_v_chunk(sc + 1)
                        if prev_norm is not None and sc < 4:
                            ptr_, ppo, precs = prev_norm
                            for k in (2 * sc, 2 * sc + 1):
                                emit_norm_unit(ptr_, k // 4, k % 4, ppo[k // 4], precs[k // 4])
                        if tr > 0 and 6 <= sc < 14:
                            idx = sc - 6
                            emit_z_tile(tr - 1, idx // 2, idx % 2)
                        if qtr < TR_N:
                            if sc == 2:
                                emit_q_chain(qtr, 0, xq_next)
                            elif sc == 4:
                                emit_q_copy(0)
                            elif sc == 5:
                                emit_q_chain(qtr, 1, xq_next)
                            elif sc == 7:
                                emit_q_copy(1)
                        if tr + 1 < TR_N:
                            if sc == 6:
                                load_mslab_half(tr + 1, 0)
                            elif sc == 8:
                                load_mslab_half(tr + 1, 1)
                            elif sc == 12 and tr + 2 < TR_N:
                                # prefetch next range's xq slab
                                xq_next = xqpool.tile(
                                    [128, CK_N, 512], F16, tag="xq", name=f"xqs{tr + 2}"
                                )
                                nc.sync.dma_start(xq_next[:], xq[tr + 2])
                    # flush pending AVs/dens, then reciprocals (frees dd).
                    # In the last range, flush hp0's dens first so its norm
                    # and transposes start before hp1's AVs finish.
                    last = tr == TR_N - 1
                    while pend_av:
                        emit_av(pend_av.pop(0))
                    recs = [None, None]
                    for hp in range(2):
                        rest = []
                        for st in pend_den:
                            if last and st[1] != hp:
                                rest.append(st)
                            else:
                                emit_den(st)
                        pend_den = rest
                        rec = rpool.tile([128, 8], F32, tag="rec")
                        nc.vector.reciprocal(rec[:], dd_t[:, hp * 8 : hp * 8 + 8])
                        recs[hp] = rec
                        if last:
                            for tcl in range(4):
                                emit_norm_unit(tr, hp, tcl, po_hp[hp], rec)
                        if not last:
                            break
                    if not last:
                        rec = rpool.tile([128, 8], F32, tag="rec")
                        nc.vector.reciprocal(rec[:], dd_t[:, 8:16])
                        recs[1] = rec
                    if not last:
                        # snapshot po to SBUF f16 before the next range's AVs
                        # recycle the PSUM buffers (deferred norm reads this)
                        pc = []
                        for hp in range(2):
                            c = pcache.tile([128, 512], F16, tag="pc", name=f"pc{tr}_{hp}")
                            nc.vector.tensor_copy(c[:], po_hp[hp][:])
                            pc.append(c)
                        prev_norm = (tr, pc, recs)
                    else:
                        prev_norm = (tr, po_hp, recs)
                # ---- tail: Z via the freed s-pool (2-deep) ----
                ltr, lpo, lrecs = prev_norm
                for tcl in range(4):
                    tcc = ltr * 4 + tcl
                    psz = spool.tile([128, 1024], F32, tag="s", name=f"zt{tcl}")
                    for er in range(ER_N):
                        for jk in range(JK_N):
                            nc.tensor.matmul(
                                psz[:, er * 512 : (er + 1) * 512],
                                oT[:, jk, ts(tcc, 128)],
                                wp_sb[:, jk, ts(er, 512)],
                                start=(jk == 0),
                                stop=(jk == JK_N - 1),
                            )
                    for er in range(ER_N):
                        z_t = zoutp.tile([128, 512], F16, tag="zt")
                        if er == 0:
                            nc.vector.tensor_copy(z_t[:], psz[:, 0:512])
                        else:
                            nc.scalar.copy(z_t[:], psz[:, 512:1024])
                        nc.sync.dma_start(zp[tcc, er], z_t[:])

            mpool_cm.__exit__(None, None, None)

    nc.compile()
    return nc


_NC = None


def _get_nc():
    global _NC
    if _NC is None:
        _NC = build_core_program()
    return _NC


def make_in_maps(inputs):
    x_q = np.asarray(inputs["x_q"], np.float32)
    x_r = np.asarray(inputs["x_r"], np.float32)
    y = np.asarray(inputs["y"], np.float32)
    mask = np.asarray(inputs["mask"])
    dist = np.asarray(inputs["dist"], np.float32)
    Wq, bq, Wk, bk, Wv, bv, Wp, bp = (
        np.asarray(inputs[k], np.float32)
        for k in ("Wq", "bq", "Wk", "bk", "Wv", "bv", "Wp", "bp")
    )

    s = np.float32(1.0 / np.sqrt(HD))

    per_batch = []
    for b in range(B):
        maskf = (mask[b, 0] != 0).astype(np.float32)  # [T1, T2]
        dmod = np.exp(-np.square(dist[b, 0] / GAMMA)).astype(np.float32)

        # [s, t] tiled as [tr, 128(s-part), sc, 512(t)]
        def tile_st(a):
            return np.ascontiguousarray(
                a.T.reshape(SC_N, 128, TR_N, 512).transpose(2, 1, 0, 3)
            ).astype(np.float16)

        mfT_ = tile_st(maskf)
        ffT_ = tile_st(maskf * dmod)

        # x^T [c, t] -> [tr/quarter 4, 128, ck, 512]
        def tile_x(a):
            aT = a.T.reshape(CK_N, 128, 4, 512)
            return np.ascontiguousarray(aT.transpose(2, 1, 0, 3)).astype(np.float16)

        xqT_ = tile_x(x_q[b])
        xrT_ = tile_x(x_r[b])
        # y[:, b] [NI, T2, C] -> [sc, 128p(c), NI, ck, 128(s)]
        yb = y[:, b].reshape(NI, SC_N, 128, CK_N, 128)  # i, sc, sl, ck, p
        yT_ = np.ascontiguousarray(yb.transpose(1, 4, 0, 3, 2)).astype(np.float16)
        per_batch.append((xqT_, xrT_, yT_, mfT_, ffT_))

    in_maps = []
    for core in range(NCORES):
        b, hg = divmod(core, HG)
        sl = slice(hg * W, (hg + 1) * W)
        xqT_, xrT_, yT_, mfT_, ffT_ = per_batch[b]
        wq_ = (Wq[:, sl] * s).reshape(CK_N, 128, JK_N, 128).transpose(1, 0, 2, 3)
        wk_ = Wk[:, sl].reshape(CK_N, 128, JK_N, 128).transpose(1, 0, 2, 3)
        wv_ = Wv[:, :, sl].reshape(NI, CK_N, 128, W).transpose(2, 0, 1, 3)
        wp_ = Wp[sl, :].reshape(JK_N, 128, C).transpose(1, 0, 2)
        in_maps.append(
            {
                "xqT": xqT_,
                "xrT": xrT_,
                "yT": yT_,
                "mfT": mfT_,
                "ffT": ffT_,
                "wq": np.ascontiguousarray(wq_).astype(np.float16),
                "wk": np.ascontiguousarray(wk_).astype(np.float16),
                "wv": np.ascontiguousarray(wv_).astype(np.float16),
                "wp": np.ascontiguousarray(wp_).astype(np.float16),
                "bq2": np.ascontiguousarray((bq[sl] * s).reshape(JK_N, 128).T).astype(np.float32),
                "bk2": np.ascontiguousarray(bk[sl].reshape(JK_N, 128).T).astype(np.float32),
                "bvs": bv.sum(0)[sl].reshape(1, W).astype(np.float16),
                "ones_r": np.ones((1, 128), np.float16),
                "ones_c": np.ones((128, 1), np.float16),
            }
        )
    return in_maps


def kernel(x_q, x_r, y, mask, dist, Wq, bq, Wk, bk, Wv, bv, Wp, bp):
    inputs = dict(
        x_q=x_q, x_r=x_r, y=y, mask=mask, dist=dist,
        Wq=Wq, bq=bq, Wk=Wk, bk=bk, Wv=Wv, bv=bv, Wp=Wp, bp=bp,
    )
    in_maps = make_in_maps(inputs)
    nc = _get_nc()
    last = None
    for _ in range(3):
        try:
            res = run_bass_kernel_spmd(nc, in_maps, list(range(NCORES)))
            break
        except Exception as e:  # transient NRT device errors: retry
            last = e
    else:
        raise last

    out = np.zeros((B, T1, C), np.float32)
    for core in range(NCORES):
        b = core // HG
        z = res.results[core]["zpart"]  # [tcc, er, 128, 512]
        out[b] += z.astype(np.float32).transpose(0, 2, 1, 3).reshape(T1, C)
    out += np.asarray(bp, np.float32)[None, None, :]
    return out
